# revision 4
# baseline (speedup 1.0000x reference)
"""Trainium2 Bass kernel for multi-head attention with RoPE (causal).

Contract: kernel(**inputs) takes FULL unsharded inputs
  x (B,T,C) f32, w_qkv (3C,C), b_qkv (3C,), w_out (C,C), b_out (C,)
and returns the FULL (B,T,C) f32 output.

Sharding: heads are split across 8 NeuronCores (tensor parallel, 2 heads
per core). Each core computes its heads' attention and a partial output
projection over its 256 columns of att_v; the host sums the 8 partials
(the "all-reduce") and adds the bias constant.

All matmuls run in float32r (full-rate fp32 mode on the PE).
"""

import sys, math
sys.path.insert(0, "/opt/trn_rl_repo")
import numpy as np
from contextlib import ExitStack

import concourse.bass as bass  # noqa: F401
import concourse.tile as tile
from concourse import bacc, mybir
from concourse.bass_utils import run_bass_kernel_spmd

F32 = mybir.dt.float32
F32R = mybir.dt.float32r
AF = mybir.ActivationFunctionType

NUM_HEADS = 16
BASE = 10000.0
N_CORES = 8
C = 2048
D = 128


def build_nc(B, T, HPC):
    """One core's program: HPC heads, all B batches, full T."""
    CH = 512                 # t-chunk (query block)
    NCC = C // 128           # c-chunks in the contraction dim
    NTC = T // CH            # t-chunks
    NTT = T // 128           # t-tiles
    NJ = 2 * HPC             # q,k d-tiles (q_h0, q_h1, k_h0, k_h1)
    NV = HPC * D             # v columns per core
    WCOLS = 3 * HPC * D      # packed W columns
    SCALE = float(1.0 / math.sqrt(D))

    nc = bacc.Bacc("TRN2", target_bir_lowering=False, debug=False,
                   enable_asserts=False)
    xtd = nc.dram_tensor("xt", [B, C, T], F32R, kind="ExternalInput").ap()
    wqd = nc.dram_tensor("wq", [NCC, 128, WCOLS], F32R, kind="ExternalInput").ap()
    wod = nc.dram_tensor("wo", [HPC, D, C], F32R, kind="ExternalInput").ap()
    cosd = nc.dram_tensor("cos2", [D, T], F32, kind="ExternalInput").ap()
    sind = nc.dram_tensor("sin2", [D, T], F32, kind="ExternalInput").ap()
    pimd = nc.dram_tensor("pim", [D, D], F32R, kind="ExternalInput").ap()
    bqkd = nc.dram_tensor("bqk", [128, NJ], F32, kind="ExternalInput").ap()
    onecd = nc.dram_tensor("onec", [128, 1], F32R, kind="ExternalInput").ap()
    onerd = nc.dram_tensor("oner", [1, 128], F32R, kind="ExternalInput").ap()
    outd = nc.dram_tensor("outp", [B, T, C], F32, kind="ExternalOutput").ap()

    with tile.TileContext(nc) as tc, ExitStack() as ctx, \
            nc.allow_low_precision(reason="float32r is full-width fp32 storage"):
        consts = ctx.enter_context(tc.tile_pool(name="consts", bufs=1))
        xtp = ctx.enter_context(tc.tile_pool(name="xtp", bufs=16))
        qkp = ctx.enter_context(tc.tile_pool(name="qkp", bufs=2))
        kvp = ctx.enter_context(tc.tile_pool(name="kvp", bufs=1))
        tmp = ctx.enter_context(tc.tile_pool(name="tmp", bufs=2))
        ep = ctx.enter_context(tc.tile_pool(name="ep", bufs=2))
        usp = ctx.enter_context(tc.tile_pool(name="usp", bufs=2))
        rbp = ctx.enter_context(tc.tile_pool(name="rbp", bufs=1))
        rp = ctx.enter_context(tc.tile_pool(name="rp", bufs=2))
        osp = ctx.enter_context(tc.tile_pool(name="osp", bufs=2))
        pa = ctx.enter_context(tc.tile_pool(name="pa", bufs=4, space="PSUM"))
        sp = ctx.enter_context(tc.tile_pool(name="sp", bufs=2, space="PSUM"))
        up = ctx.enter_context(tc.tile_pool(name="up", bufs=1, space="PSUM"))
        lp = ctx.enter_context(tc.tile_pool(name="lp", bufs=1, space="PSUM"))

        wq_sb = consts.tile([128, NCC, WCOLS], F32R)
        for ci in range(NCC):
            nc.sync.dma_start(out=wq_sb[:, ci, :], in_=wqd[ci])
        wo_sb = consts.tile([128, HPC, C], F32R)
        for h in range(HPC):
            nc.sync.dma_start(out=wo_sb[:, h, :], in_=wod[h])
        cos_sb = consts.tile([128, T], F32)
        nc.sync.dma_start(out=cos_sb, in_=cosd)
        sin_sb = consts.tile([128, T], F32)
        nc.sync.dma_start(out=sin_sb, in_=sind)
        pim_sb = consts.tile([128, 128], F32R)
        nc.sync.dma_start(out=pim_sb, in_=pimd)
        bqk_sb = consts.tile([128, NJ], F32)
        nc.sync.dma_start(out=bqk_sb, in_=bqkd)
        onec_sb = consts.tile([128, 1], F32R)
        nc.sync.dma_start(out=onec_sb, in_=onecd)
        oner_sb = consts.tile([1, 128], F32R)
        nc.sync.dma_start(out=oner_sb, in_=onerd)

        for b in range(B):
            k_sb = kvp.tile([128, HPC, T], F32R, tag="k")
            v_sb = kvp.tile([128, NTT, NV], F32R, tag="v")
            for tci in range(NTC):
                ts0 = tci * CH
                # ---- stream x^T slices for this chunk
                xts = []
                for ci in range(NCC):
                    xt_t = xtp.tile([128, CH], F32R, tag="xt")
                    nc.sync.dma_start(
                        out=xt_t,
                        in_=xtd[b, ci * 128:(ci + 1) * 128, ts0:ts0 + CH])
                    xts.append(xt_t)
                # ---- q,k projection (transposed layout) + RoPE
                q_t = qkp.tile([128, HPC, CH], F32R, tag="q")
                for j in range(NJ):
                    ps = pa.tile([128, CH], F32, tag="pa")
                    for ci in range(NCC):
                        nc.tensor.matmul(ps, wq_sb[:, ci, j * 128:(j + 1) * 128],
                                         xts[ci], start=(ci == 0),
                                         stop=(ci == NCC - 1))
                    raw = tmp.tile([128, CH], F32R, tag="raw")
                    nc.scalar.activation(raw, ps, AF.Identity,
                                         bias=bqk_sb[:, j:j + 1])
                    qp = pa.tile([128, CH], F32, tag="pa")
                    nc.tensor.matmul(qp, pim_sb, raw, start=True, stop=True)
                    t1 = tmp.tile([128, CH], F32, tag="t1")
                    nc.vector.tensor_mul(t1, raw, cos_sb[:, ts0:ts0 + CH])
                    t2 = tmp.tile([128, CH], F32, tag="t2")
                    nc.vector.tensor_mul(t2, qp, sin_sb[:, ts0:ts0 + CH])
                    dest = (q_t[:, j, :] if j < HPC
                            else k_sb[:, j - HPC, ts0:ts0 + CH])
                    nc.vector.tensor_add(dest, t1, t2)
                # ---- v projection (natural layout)
                for tt in range(CH // 128):
                    ps = pa.tile([128, NV], F32, tag="pa")
                    for ci in range(NCC):
                        nc.tensor.matmul(ps, xts[ci][:, tt * 128:(tt + 1) * 128],
                                         wq_sb[:, ci, NJ * 128:],
                                         start=(ci == 0), stop=(ci == NCC - 1))
                    nc.scalar.copy(v_sb[:, tci * (CH // 128) + tt, :], ps)
                # ---- attention for this chunk (all s-tiles <= chunk)
                us_t = usp.tile([128, HPC, CH], F32R, tag="us")
                ns = 4 * tci + 4
                for h in range(HPC):
                    u_ps = up.tile([128, CH], F32, tag="u")
                    l_ps = lp.tile([1, CH], F32, tag="l")
                    for si in range(ns):
                        s_ps = sp.tile([128, CH], F32, tag="s")
                        nc.tensor.matmul(s_ps, k_sb[:, h, si * 128:(si + 1) * 128],
                                         q_t[:, h, :], start=True, stop=True)
                        e_t = ep.tile([128, CH], F32R, tag="e")
                        nc.scalar.activation(e_t, s_ps, AF.Exp, scale=SCALE)
                        o = si - 4 * tci
                        if o >= 0:  # diagonal block: zero s>t entries
                            nc.gpsimd.affine_select(
                                out=e_t, in_=e_t,
                                compare_op=mybir.AluOpType.is_ge, fill=0.0,
                                base=-128 * o, pattern=[[1, CH]],
                                channel_multiplier=-1)
                        nc.tensor.matmul(u_ps, v_sb[:, si, h * 128:(h + 1) * 128],
                                         e_t, start=(si == 0), stop=(si == ns - 1))
                        nc.tensor.matmul(l_ps, onec_sb, e_t,
                                         start=(si == 0), stop=(si == ns - 1))
                    r_t = rp.tile([1, CH], F32R, tag="r")
                    nc.vector.reciprocal(r_t, l_ps)
                    rb_ps = sp.tile([128, CH], F32, tag="s")
                    nc.tensor.matmul(rb_ps, oner_sb, r_t, start=True, stop=True)
                    rb_sb = rbp.tile([128, CH], F32, tag="rb")
                    nc.scalar.copy(rb_sb, rb_ps)
                    nc.vector.tensor_mul(us_t[:, h, :], u_ps, rb_sb)
                # ---- partial out-projection for this chunk
                for tt in range(CH // 128):
                    t0 = ts0 + tt * 128
                    for half in range(2):
                        ost = osp.tile([128, 1024], F32, tag="ost")
                        for cc in range(2):
                            ps = pa.tile([128, 512], F32, tag="pa")
                            c0 = half * 1024 + cc * 512
                            for h in range(HPC):
                                nc.tensor.matmul(
                                    ps, us_t[:, h, tt * 128:(tt + 1) * 128],
                                    wo_sb[:, h, c0:c0 + 512],
                                    start=(h == 0), stop=(h == HPC - 1))
                            if cc == 0:
                                nc.scalar.copy(ost[:, :512], ps)
                            else:
                                nc.vector.tensor_copy(ost[:, 512:], ps)
                        nc.sync.dma_start(
                            out=outd[b, t0:t0 + 128, half * 1024:half * 1024 + 1024],
                            in_=ost)
    nc.compile()
    return nc


def _rope_tables(T):
    half = D // 2
    thetas = BASE ** (-np.arange(half, dtype=np.float32) / half)
    ang = np.arange(T, dtype=np.float32)[:, None] * thetas[None, :]  # (T, half)
    sin = np.sin(ang).astype(np.float32)
    cos = np.cos(ang).astype(np.float32)
    # duplicate per pair along d: table[d, t] = f(t, d//2)
    sin2 = np.repeat(sin.T, 2, axis=0)  # (D, T)
    cos2 = np.repeat(cos.T, 2, axis=0)
    return np.ascontiguousarray(sin2), np.ascontiguousarray(cos2)


def _pi_matrix():
    # qp = PI @ q with qp[2i] = -q[2i+1], qp[2i+1] = q[2i]; matmul takes PI^T
    pim = np.zeros((D, D), dtype=np.float32)
    for i in range(D // 2):
        pim[2 * i + 1, 2 * i] = -1.0
        pim[2 * i, 2 * i + 1] = 1.0
    return pim


_NC_CACHE = {}


def _get_nc(B, T, HPC):
    key = (B, T, HPC)
    if key not in _NC_CACHE:
        _NC_CACHE[key] = build_nc(B, T, HPC)
    return _NC_CACHE[key]


def make_in_maps(x, w_qkv, b_qkv, n_cores=N_CORES, hpc=None):
    B, T, Cx = x.shape
    assert Cx == C
    HPC = hpc if hpc is not None else NUM_HEADS // n_cores
    xt = np.ascontiguousarray(np.transpose(x, (0, 2, 1)))  # (B, C, T)
    sin2, cos2 = _rope_tables(T)
    pim = _pi_matrix()
    onec = np.ones((128, 1), dtype=np.float32)
    oner = np.ones((1, 128), dtype=np.float32)
    in_maps = []
    for c in range(n_cores):
        heads = [c * HPC + h for h in range(HPC)]
        rows = np.concatenate(
            [np.arange(h * D, (h + 1) * D) for h in heads] +           # q
            [np.arange(C + h * D, C + (h + 1) * D) for h in heads] +   # k
            [np.arange(2 * C + h * D, 2 * C + (h + 1) * D) for h in heads])  # v
        wq = np.ascontiguousarray(w_qkv[rows].T).reshape(C // 128, 128, 3 * HPC * D)
        bq = b_qkv[rows[:2 * HPC * D]].reshape(2 * HPC, D).T  # (128, NJ)
        in_maps.append({
            "xt": xt,
            "wq": np.ascontiguousarray(wq, dtype=np.float32),
            "bqk": np.ascontiguousarray(bq, dtype=np.float32),
            "cos2": cos2,
            "sin2": sin2,
            "pim": pim,
            "onec": onec,
            "oner": oner,
        })
    return in_maps


def kernel(x, w_qkv, b_qkv, w_out, b_out):
    x = np.asarray(x, dtype=np.float32)
    w_qkv = np.asarray(w_qkv, dtype=np.float32)
    b_qkv = np.asarray(b_qkv, dtype=np.float32)
    w_out = np.asarray(w_out, dtype=np.float32)
    b_out = np.asarray(b_out, dtype=np.float32)
    B, T, Cx = x.shape
    HPC = NUM_HEADS // N_CORES
    nc = _get_nc(B, T, HPC)

    in_maps = make_in_maps(x, w_qkv, b_qkv, N_CORES)
    for c in range(N_CORES):
        heads = [c * HPC + h for h in range(HPC)]
        wo = np.stack([np.ascontiguousarray(w_out[:, h * D:(h + 1) * D].T)
                       for h in heads])  # (HPC, D, C)
        in_maps[c]["wo"] = wo

    res = run_bass_kernel_spmd(nc, in_maps, core_ids=list(range(N_CORES)))
    out = np.zeros((B, T, C), dtype=np.float64)
    for c in range(N_CORES):
        out += res.results[c]["outp"]
    b_v = b_qkv[2 * C:]
    const = w_out.astype(np.float64) @ b_v.astype(np.float64) + b_out
    out += const[None, None, :]
    return out.astype(np.float32)


# revision 9
# speedup vs baseline: 1.0277x; 1.0277x over previous
"""Trainium2 Bass kernel for multi-head attention with RoPE (causal).

Contract: kernel(**inputs) takes FULL unsharded inputs
  x (B,T,C) f32, w_qkv (3C,C), b_qkv (3C,), w_out (C,C), b_out (C,)
and returns the FULL (B,T,C) f32 output.

Sharding: heads are split across 8 NeuronCores (tensor parallel, 2 heads
per core). Each core computes its heads' attention and a partial output
projection over its 256 columns of att_v; the host sums the 8 partials
(the "all-reduce") and adds the bias constant.

All matmuls run in float32r (full-rate fp32 mode on the PE).
"""

import sys, math
sys.path.insert(0, "/opt/trn_rl_repo")
import numpy as np
from contextlib import ExitStack

import concourse.bass as bass  # noqa: F401
import concourse.tile as tile
from concourse import bacc, mybir
from concourse.bass_utils import run_bass_kernel_spmd

F32 = mybir.dt.float32
F32R = mybir.dt.float32r
AF = mybir.ActivationFunctionType

NUM_HEADS = 16
BASE = 10000.0
N_CORES = 8
C = 2048
D = 128


def build_nc(B, T, HPC):
    """One core's program: HPC heads, all B batches, full T."""
    CH = 512                 # t-chunk (query block)
    NCC = C // 128           # c-chunks in the contraction dim
    NTC = T // CH            # t-chunks
    NTT = T // 128           # t-tiles
    NJ = 2 * HPC             # q,k d-tiles (q_h0, q_h1, k_h0, k_h1)
    NV = HPC * D             # v columns per core
    WCOLS = 3 * HPC * D      # packed W columns
    SCALE = float(1.0 / math.sqrt(D))

    nc = bacc.Bacc("TRN2", target_bir_lowering=False, debug=False,
                   enable_asserts=False)
    xtd = nc.dram_tensor("xt", [B, C, T], F32R, kind="ExternalInput").ap()
    wqd = nc.dram_tensor("wq", [NCC, 128, WCOLS], F32R, kind="ExternalInput").ap()
    wod = nc.dram_tensor("wo", [HPC, D, C], F32R, kind="ExternalInput").ap()
    cosd = nc.dram_tensor("cos2", [D, T], F32, kind="ExternalInput").ap()
    sind = nc.dram_tensor("sin2", [D, T], F32, kind="ExternalInput").ap()
    pimd = nc.dram_tensor("pim", [D, D], F32R, kind="ExternalInput").ap()
    bqkd = nc.dram_tensor("bqk", [128, NJ], F32, kind="ExternalInput").ap()
    onecd = nc.dram_tensor("onec", [128, 1], F32R, kind="ExternalInput").ap()
    onerd = nc.dram_tensor("oner", [1, 128], F32R, kind="ExternalInput").ap()
    outd = nc.dram_tensor("outp", [B, T, C], F32, kind="ExternalOutput").ap()

    with tile.TileContext(nc) as tc, ExitStack() as ctx, \
            nc.allow_low_precision(reason="float32r is full-width fp32 storage"):
        consts = ctx.enter_context(tc.tile_pool(name="consts", bufs=1))
        xtp = ctx.enter_context(tc.tile_pool(name="xtp", bufs=16))
        qkp = ctx.enter_context(tc.tile_pool(name="qkp", bufs=2))
        kvp = ctx.enter_context(tc.tile_pool(name="kvp", bufs=1))
        tmp = ctx.enter_context(tc.tile_pool(name="tmp", bufs=2))
        ep = ctx.enter_context(tc.tile_pool(name="ep", bufs=2))
        usp = ctx.enter_context(tc.tile_pool(name="usp", bufs=2))
        rbp = ctx.enter_context(tc.tile_pool(name="rbp", bufs=1))
        rp = ctx.enter_context(tc.tile_pool(name="rp", bufs=2))
        osp = ctx.enter_context(tc.tile_pool(name="osp", bufs=2))
        pa = ctx.enter_context(tc.tile_pool(name="pa", bufs=3, space="PSUM"))
        sp = ctx.enter_context(tc.tile_pool(name="sp", bufs=2, space="PSUM"))
        up = ctx.enter_context(tc.tile_pool(name="up", bufs=2, space="PSUM"))
        lp = ctx.enter_context(tc.tile_pool(name="lp", bufs=1, space="PSUM"))

        wq_sb = consts.tile([128, NCC, WCOLS], F32R)
        for ci in range(NCC):
            nc.sync.dma_start(out=wq_sb[:, ci, :], in_=wqd[ci])
        wo_sb = consts.tile([128, HPC, C], F32R)
        for h in range(HPC):
            nc.sync.dma_start(out=wo_sb[:, h, :], in_=wod[h])
        cos_sb = consts.tile([128, T], F32)
        nc.sync.dma_start(out=cos_sb, in_=cosd)
        sin_sb = consts.tile([128, T], F32)
        nc.sync.dma_start(out=sin_sb, in_=sind)
        pim_sb = consts.tile([128, 128], F32R)
        nc.sync.dma_start(out=pim_sb, in_=pimd)
        bqk_sb = consts.tile([128, NJ], F32)
        nc.sync.dma_start(out=bqk_sb, in_=bqkd)
        onec_sb = consts.tile([128, 1], F32R)
        nc.sync.dma_start(out=onec_sb, in_=onecd)
        oner_sb = consts.tile([33, 128], F32R)
        nc.sync.dma_start(out=oner_sb[0:1, :], in_=onerd)
        nc.sync.dma_start(out=oner_sb[32:33, :], in_=onerd)

        for b in range(B):
            k_sb = kvp.tile([128, HPC, T], F32R, tag="k")
            v_sb = kvp.tile([128, NTT, NV], F32R, tag="v")
            for tci in range(NTC):
                ts0 = tci * CH
                # ---- stream x^T slices for this chunk
                xts = []
                for ci in range(NCC):
                    xt_t = xtp.tile([128, CH], F32R, tag="xt")
                    nc.sync.dma_start(
                        out=xt_t,
                        in_=xtd[b, ci * 128:(ci + 1) * 128, ts0:ts0 + CH])
                    xts.append(xt_t)
                # ---- q,k projection (transposed layout) + RoPE
                q_t = qkp.tile([128, HPC, CH], F32R, tag="q")
                for j in range(NJ):
                    ps = pa.tile([128, CH], F32, tag="pa")
                    for ci in range(NCC):
                        nc.tensor.matmul(ps, wq_sb[:, ci, j * 128:(j + 1) * 128],
                                         xts[ci], start=(ci == 0),
                                         stop=(ci == NCC - 1))
                    raw = tmp.tile([128, CH], F32R, tag="raw")
                    nc.scalar.activation(raw, ps, AF.Identity,
                                         bias=bqk_sb[:, j:j + 1])
                    qp = pa.tile([128, CH], F32, tag="pa")
                    nc.tensor.matmul(qp, pim_sb, raw, start=True, stop=True)
                    t1 = tmp.tile([128, CH], F32, tag="t1")
                    nc.vector.tensor_mul(t1, raw, cos_sb[:, ts0:ts0 + CH])
                    t2 = tmp.tile([128, CH], F32, tag="t2")
                    nc.vector.tensor_mul(t2, qp, sin_sb[:, ts0:ts0 + CH])
                    dest = (q_t[:, j, :] if j < HPC
                            else k_sb[:, j - HPC, ts0:ts0 + CH])
                    nc.vector.tensor_add(dest, t1, t2)
                # ---- v projection (natural layout)
                for tt in range(CH // 128):
                    ps = pa.tile([128, NV], F32, tag="pa")
                    for ci in range(NCC):
                        nc.tensor.matmul(ps, xts[ci][:, tt * 128:(tt + 1) * 128],
                                         wq_sb[:, ci, NJ * 128:],
                                         start=(ci == 0), stop=(ci == NCC - 1))
                    nc.scalar.copy(v_sb[:, tci * (CH // 128) + tt, :], ps)
                # ---- attention for this chunk (all s-tiles <= chunk)
                us_t = usp.tile([128, HPC, CH], F32R, tag="us")
                l2_sb = rp.tile([32 * (HPC - 1) + 1, CH], F32, tag="l2")
                u_list = []
                ns = 4 * tci + 4
                for h in range(HPC):
                    u_ps = up.tile([128, CH], F32, tag="u")
                    l_ps = lp.tile([1, CH], F32, tag="l")
                    for si in range(ns):
                        s_ps = sp.tile([128, CH], F32, tag="s")
                        nc.tensor.matmul(s_ps, k_sb[:, h, si * 128:(si + 1) * 128],
                                         q_t[:, h, :], start=True, stop=True)
                        e_t = ep.tile([128, CH], F32R, tag="e")
                        nc.scalar.activation(e_t, s_ps, AF.Exp, scale=SCALE)
                        o = si - 4 * tci
                        if o >= 0:  # diagonal block: zero s>t entries
                            nc.gpsimd.affine_select(
                                out=e_t, in_=e_t,
                                compare_op=mybir.AluOpType.is_ge, fill=0.0,
                                base=-128 * o, pattern=[[1, CH]],
                                channel_multiplier=-1)
                        nc.tensor.matmul(u_ps, v_sb[:, si, h * 128:(h + 1) * 128],
                                         e_t, start=(si == 0), stop=(si == ns - 1))
                        nc.tensor.matmul(l_ps, onec_sb, e_t,
                                         start=(si == 0), stop=(si == ns - 1))
                    nc.scalar.copy(l2_sb[32 * h:32 * h + 1, :], l_ps)
                    u_list.append(u_ps)
                r_t = rp.tile([32 * (HPC - 1) + 1, CH], F32R, tag="r")
                nc.vector.reciprocal(r_t, l2_sb)
                for h in range(HPC):
                    rb_ps = sp.tile([128, CH], F32, tag="s")
                    nc.tensor.matmul(rb_ps, oner_sb[32 * h:32 * h + 1, :],
                                     r_t[32 * h:32 * h + 1, :],
                                     start=True, stop=True)
                    rb_sb = rbp.tile([128, CH], F32, tag="rb")
                    nc.scalar.copy(rb_sb, rb_ps)
                    nc.vector.tensor_mul(us_t[:, h, :], u_list[h], rb_sb)
                # ---- partial out-projection for this chunk
                for tt in range(CH // 128):
                    t0 = ts0 + tt * 128
                    for half in range(2):
                        ost = osp.tile([128, 1024], F32, tag="ost")
                        pss = [pa.tile([128, 512], F32, tag="pa",
                                       name=f"ops{i}") for i in range(2)]
                        for h in range(HPC):
                            for cc in range(2):
                                c0 = half * 1024 + cc * 512
                                nc.tensor.matmul(
                                    pss[cc], us_t[:, h, tt * 128:(tt + 1) * 128],
                                    wo_sb[:, h, c0:c0 + 512],
                                    start=(h == 0), stop=(h == HPC - 1))
                        nc.scalar.copy(ost[:, :512], pss[0])
                        nc.vector.tensor_copy(ost[:, 512:], pss[1])
                        nc.sync.dma_start(
                            out=outd[b, t0:t0 + 128, half * 1024:half * 1024 + 1024],
                            in_=ost)
    nc.compile()
    return nc


def _rope_tables(T):
    half = D // 2
    thetas = BASE ** (-np.arange(half, dtype=np.float32) / half)
    ang = np.arange(T, dtype=np.float32)[:, None] * thetas[None, :]  # (T, half)
    sin = np.sin(ang).astype(np.float32)
    cos = np.cos(ang).astype(np.float32)
    # duplicate per pair along d: table[d, t] = f(t, d//2)
    sin2 = np.repeat(sin.T, 2, axis=0)  # (D, T)
    cos2 = np.repeat(cos.T, 2, axis=0)
    return np.ascontiguousarray(sin2), np.ascontiguousarray(cos2)


def _pi_matrix():
    # qp = PI @ q with qp[2i] = -q[2i+1], qp[2i+1] = q[2i]; matmul takes PI^T
    pim = np.zeros((D, D), dtype=np.float32)
    for i in range(D // 2):
        pim[2 * i + 1, 2 * i] = -1.0
        pim[2 * i, 2 * i + 1] = 1.0
    return pim


_NC_CACHE = {}


def _get_nc(B, T, HPC):
    key = (B, T, HPC)
    if key not in _NC_CACHE:
        _NC_CACHE[key] = build_nc(B, T, HPC)
    return _NC_CACHE[key]


def make_in_maps(x, w_qkv, b_qkv, n_cores=N_CORES, hpc=None):
    B, T, Cx = x.shape
    assert Cx == C
    HPC = hpc if hpc is not None else NUM_HEADS // n_cores
    xt = np.ascontiguousarray(np.transpose(x, (0, 2, 1)))  # (B, C, T)
    sin2, cos2 = _rope_tables(T)
    pim = _pi_matrix()
    onec = np.ones((128, 1), dtype=np.float32)
    oner = np.ones((1, 128), dtype=np.float32)
    in_maps = []
    for c in range(n_cores):
        heads = [c * HPC + h for h in range(HPC)]
        rows = np.concatenate(
            [np.arange(h * D, (h + 1) * D) for h in heads] +           # q
            [np.arange(C + h * D, C + (h + 1) * D) for h in heads] +   # k
            [np.arange(2 * C + h * D, 2 * C + (h + 1) * D) for h in heads])  # v
        wq = np.ascontiguousarray(w_qkv[rows].T).reshape(C // 128, 128, 3 * HPC * D)
        bq = b_qkv[rows[:2 * HPC * D]].reshape(2 * HPC, D).T  # (128, NJ)
        in_maps.append({
            "xt": xt,
            "wq": np.ascontiguousarray(wq, dtype=np.float32),
            "bqk": np.ascontiguousarray(bq, dtype=np.float32),
            "cos2": cos2,
            "sin2": sin2,
            "pim": pim,
            "onec": onec,
            "oner": oner,
        })
    return in_maps


def kernel(x, w_qkv, b_qkv, w_out, b_out):
    x = np.asarray(x, dtype=np.float32)
    w_qkv = np.asarray(w_qkv, dtype=np.float32)
    b_qkv = np.asarray(b_qkv, dtype=np.float32)
    w_out = np.asarray(w_out, dtype=np.float32)
    b_out = np.asarray(b_out, dtype=np.float32)
    B, T, Cx = x.shape
    HPC = NUM_HEADS // N_CORES
    nc = _get_nc(B, T, HPC)

    in_maps = make_in_maps(x, w_qkv, b_qkv, N_CORES)
    for c in range(N_CORES):
        heads = [c * HPC + h for h in range(HPC)]
        wo = np.stack([np.ascontiguousarray(w_out[:, h * D:(h + 1) * D].T)
                       for h in heads])  # (HPC, D, C)
        in_maps[c]["wo"] = wo

    res = run_bass_kernel_spmd(nc, in_maps, core_ids=list(range(N_CORES)))
    out = np.zeros((B, T, C), dtype=np.float64)
    for c in range(N_CORES):
        out += res.results[c]["outp"]
    b_v = b_qkv[2 * C:]
    const = w_out.astype(np.float64) @ b_v.astype(np.float64) + b_out
    out += const[None, None, :]
    return out.astype(np.float32)
